# revision 71
# baseline (speedup 1.0000x reference)
"""AttentionPool kernel for Trainium2, 8 NeuronCores (SPMD data-parallel).

Reference computation (per graph g with atoms A_g, uniform |A_g| = 32):
    h = X @ W.T                              [131072, 512]
    s = leakyrelu(sum(att * h, -1), 0.2)     [131072]
    w = segment_softmax(s)                   per graph
    out[g] = sum_{a in A_g} w[a] * h[a]      [4096, 512]

Algebraic refactor (pool-first; avoids the 69-GFLOP h matmul):
    v  = W.T @ att   (host, tiny)
    s  = lrelu(X @ v)         per-tile dot products, 3-way engine split
    e  = exp(s)               ACT
    P[b] = E_b^T X_b          PE: per 128-atom tile a [128,64] stationary
                              slice of a zero-padded block matrix holding
                              e-values at block-diagonal slots; 16 tiles
                              accumulate a [64,512] batch in one PSUM bank
    d  = E_c^T 1              a contiguous copy of E vs a ones column
    pooled = P/d              folded into the ACT PSUM->SBUF copy (scale=1/d)
    out = pooled @ W.T        PE transposes + 4 chunk matmuls per 128 graphs

Everything is fp16 on the wire and in the PE (fp32 PSUM accumulate): the PE
runs 4x faster than fp32 (1 cycle/row), DMA traffic halves (16.8MB/core),
and fp16's 11-bit mantissa keeps rel err ~1e-3 (gate is 2e-2).

The score dot products are the engine bottleneck (8.4M mul+acc per core, no
DVE fast modes for reducing ops, and GPSIMD has no free-axis reduce at all).
They are split two ways, all sharing one SBUF X tile:
  'd': DVE scalar_tensor_tensor with accum_out            (DVE ~0.65us/tile)
  't': DVE tensor_tensor product (2x mode) -> ACT Copy+accum (DVE 0.38,
       ACT 0.91 incl the 278ns accumulator read)
Emission is software-pipelined one batch deep so score ops for batch i+1
never queue behind batch i's PE-dependent copies.

X streams in as 8 uneven-span DMAs over a host-packed [partition, tile,
feat] layout (first span is 4 tiles so scoring starts ~2us in); adding DMA
instructions to the sync ring costs more in SP semaphore churn than any
overlap it buys, so spans are few and fat.

The last batch's lrelu/exp/E writes are issued per 4-tile quarter so the
PE's pool matmuls (range-tracked against the E matrix) start draining
while the final tiles still score.

Measured on hardware: 97.7us exec (cool device; ~116 when the part is
power-throttled after continuous benching), rel err 6.5e-4 (baseline fp32
kernel: 189-195us at 4.1e-6; gate 2e-2). Engine busy at ~103us wall:
DVE ~93, ACT ~91, PE ~79 (mid p-state), DMA ~53 for 18MB.

Sharding: 8 cores x 16384 atoms (= 512 graphs, graph-aligned). W/att
replicated. X is host-packed fp16 in DMA-friendly [block, partition, tile,
feat] order. Non-uniform segment sizes fall back to an exact numpy path
(never triggered by the fixed harness inputs).
"""

import numpy as np

N_ATOMS = 131072
FEAT = 512
N_GRAPHS = 4096
NEG_SLOPE = 0.2
N_CORES = 8

P = 128                      # partitions / atoms per tile
NA_CORE = N_ATOMS // N_CORES         # 16384 atoms per core
NT = NA_CORE // P                    # 128 tiles per core
NG_CORE = N_GRAPHS // N_CORES        # 512 graphs per core
GPT = P // 32                        # 4 graphs per tile
TPB = 16                             # tiles per batch
GPB = GPT * TPB                      # 64 graphs per batch (1 PSUM bank)
NB = NT // TPB                       # 8 batches per core
BPG = 2                              # batches per group (128 graphs)
NGRP = NB // BPG                     # 4 groups per core
FCH = FEAT // P                      # 4 feature chunks
NDMA = 8                             # X DMA blocks per core
TPD = NT // NDMA                     # 16 tiles per DMA block
EBW = 68                             # cols per EB sub-stationary block (64+pad)
EBSTR = EBW + GPT                    # 72: diagonal slots sit at 72k+j, so one
                                     # strided [p,16(72),4(1)] write fills them
EBCOLS = TPB * EBSTR                 # 1152

# score-class tile counts (see module docstring): must sum to NT.
# GPSIMD is deliberately absent: concurrent Pool-engine tensor ops starve
# the DVE of SBUF bandwidth (measured 732 -> 1663 ns per STT).
N_CLASS_D = 68                       # DVE STT + accum
N_CLASS_G = 0                        # GPSIMD product -> ACT reduce (disabled)
N_CLASS_T = NT - N_CLASS_D - N_CLASS_G   # DVE TT product -> ACT reduce


def _score_classes():
    """Bresenham-spread the three classes evenly over the 128 tiles."""
    cnt = {c: n for c, n in
           (("d", N_CLASS_D), ("g", N_CLASS_G), ("t", N_CLASS_T)) if n > 0}
    acc = dict.fromkeys(cnt, 0.0)
    seq = []
    for _ in range(NT):
        for c in cnt:
            acc[c] += cnt[c] / NT
        pick = max(acc, key=lambda c: acc[c])
        acc[pick] -= 1.0
        seq.append(pick)
    return seq


_CACHED = {}


def _build_program():
    import concourse.bacc as bacc
    import concourse.mybir as mybir
    import concourse.tile as tile
    from contextlib import ExitStack

    F32 = mybir.dt.float32
    F16 = mybir.dt.float16
    F8 = mybir.dt.float8e4
    MULT = mybir.AluOpType.mult
    ADD = mybir.AluOpType.add
    MAX = mybir.AluOpType.max
    AXX = mybir.AxisListType.X
    EXP = mybir.ActivationFunctionType.Exp
    COPY = mybir.ActivationFunctionType.Copy
    classes = _score_classes()

    nc = bacc.Bacc("TRN2", target_bir_lowering=False, debug=False,
                   num_devices=N_CORES)

    x = nc.dram_tensor("x", [P, NT * FEAT], F16, kind="ExternalInput").ap()
    wt = nc.dram_tensor("wt", [P, FCH, FEAT], F16, kind="ExternalInput").ap()
    vrep = nc.dram_tensor("vrep", [P, FEAT], F16, kind="ExternalInput").ap()
    mask4 = nc.dram_tensor("mask4", [P, GPT], F16, kind="ExternalInput").ap()
    ident = nc.dram_tensor("ident", [GPB, GPB], F16, kind="ExternalInput").ap()
    zeros = nc.dram_tensor("zeros", [P, NB * EBCOLS], F16,
                           kind="ExternalInput").ap()
    out = nc.dram_tensor("out", [NGRP, P, FEAT], F16, kind="ExternalOutput").ap()

    with tile.TileContext(nc) as tc, ExitStack() as ctx:
        singles = ctx.enter_context(tc.tile_pool(name="singles", bufs=1))
        spool = ctx.enter_context(tc.tile_pool(name="spool", bufs=3))
        epool = ctx.enter_context(tc.tile_pool(name="epool", bufs=3))
        ecpool = ctx.enter_context(tc.tile_pool(name="ecpool", bufs=3))
        jdpool = ctx.enter_context(tc.tile_pool(name="jdpool", bufs=2))
        japool = ctx.enter_context(tc.tile_pool(name="japool", bufs=2))
        prpool = ctx.enter_context(tc.tile_pool(name="prpool", bufs=4))
        drpool = ctx.enter_context(tc.tile_pool(name="drpool", bufs=3))
        plpool = ctx.enter_context(tc.tile_pool(name="plpool", bufs=3))
        ptsb = ctx.enter_context(tc.tile_pool(name="ptsb", bufs=2))
        outp = ctx.enter_context(tc.tile_pool(name="outp", bufs=2))
        ps_bp = ctx.enter_context(tc.tile_pool(name="ps_bp", bufs=2, space="PSUM"))
        ps_den = ctx.enter_context(tc.tile_pool(name="ps_den", bufs=2, space="PSUM"))
        ps_pt = ctx.enter_context(tc.tile_pool(name="ps_pt", bufs=2, space="PSUM"))
        ps_out = ctx.enter_context(tc.tile_pool(name="ps_out", bufs=2, space="PSUM"))

        # ---- weights + X streaming (sync DGE ring, in priority order) ----
        # separate v copies per reading engine to spread SBUF contention
        v_rep = singles.tile([P, FEAT], F16)
        nc.sync.dma_start(out=v_rep, in_=vrep)
        v_gp = singles.tile([P, FEAT], F16)
        nc.sync.dma_start(out=v_gp, in_=vrep)
        mask4_sb = singles.tile([P, GPT], F16)
        nc.sync.dma_start(out=mask4_sb, in_=mask4)
        ident_sb = singles.tile([GPB, GPB], F16)
        nc.sync.dma_start(out=ident_sb, in_=ident)
        # EB holds every batch's block of 8 pool stationaries [128, 32] at
        # free offsets 36k; e-values land at flat cols 40k+j (one strided
        # write), the rest must stay zero forever. Zero the first group's
        # region on ACT (needed early); the rest arrives as DMA'd zeros so
        # ACT's score reduces aren't blocked behind a 4.6us memzero.
        eb_all = singles.tile([P, NB, EBCOLS], F16)
        nc.scalar.memzero(eb_all[:, :BPG, :])
        # X as one [P, tiles, feat] SBUF region, filled by 8 uneven span
        # DMAs (same SP instruction count as before, but the first scores
        # can start ~2us in instead of waiting for a full 2MB block)
        xbig = singles.tile([P, NT * FEAT], F16)
        spans = (4, 16, 32, 48, 64, 80, 96, NT)
        lo = 0
        for i, hi in enumerate(spans):
            nc.sync.dma_start(out=xbig[:, lo * FEAT:hi * FEAT],
                              in_=x[:, lo * FEAT:hi * FEAT])
            lo = hi
            if i == 2:
                wt_sb = singles.tile([P, FCH, FEAT], F16)
                nc.sync.dma_start(out=wt_sb, in_=wt)
            if i == 4:
                nc.sync.dma_start(
                    out=eb_all[:, BPG:, :].rearrange("p b c -> p (b c)"),
                    in_=zeros[:, BPG * EBCOLS:])

        ones_col = singles.tile([P, 1], F16)
        nc.vector.memset(ones_col, 1.0)

        # PE warmup against the HAM clock gate: busy matmuls while the
        # first batch's scores are still in flight.
        warm_ps = ps_bp.tile([GPB, FEAT], F32, tag="bp", name="warm")
        for wi in range(6):
            nc.tensor.matmul(warm_ps, lhsT=v_rep[:, :GPB], rhs=v_rep,
                             start=(wi == 0), stop=(wi == 5))

        def emit_scores(bu, split=False):
            """Scores + e-matrix builds for batch bu; returns X slices + E.

            With split=True the lrelu/exp/E writes are issued per 8-tile
            half, so the PE (whose pool matmuls track EB sub-ranges) can
            start pooling the first half while the second still scores —
            used for the last batch to shorten the drain."""
            s_b = spool.tile([P, TPB], F32, tag="s_b")
            s_lr = spool.tile([P, TPB], F32, tag="s_lr")
            e_b = epool.tile([P, TPB], F16, tag="e_b")
            econ = ecpool.tile([P, GPB], F16, tag="econ")
            diag = eb_all[:, bu, :].rearrange(
                "p (k r) -> p k r", r=EBSTR)[:, :, 0:GPT]
            econ_v = econ.rearrange("p (k c) -> p k c", c=GPT)
            m_bc = mask4_sb.unsqueeze(1)
            xts = []

            def tail(ks):
                """lrelu + exp + E-matrix writes for tile slots ks."""
                nh = ks.stop - ks.start
                nc.vector.scalar_tensor_tensor(
                    out=s_lr[:, ks], in0=s_b[:, ks], scalar=NEG_SLOPE,
                    in1=s_b[:, ks], op0=MULT, op1=MAX)
                nc.scalar.activation(out=e_b[:, ks], in_=s_lr[:, ks],
                                     func=EXP)
                e_bc = e_b[:, ks].unsqueeze(2).broadcast_to([P, nh, GPT])
                mb = m_bc.broadcast_to([P, nh, GPT])
                nc.vector.tensor_tensor(out=diag[:, ks, :], in0=e_bc,
                                        in1=mb, op=MULT)
                nc.vector.tensor_tensor(out=econ_v[:, ks, :], in0=e_bc,
                                        in1=mb, op=MULT)

            for k in range(TPB):
                t = bu * TPB + k
                xt = xbig[:, t * FEAT:(t + 1) * FEAT]
                xts.append(xt)
                acc = s_b[:, k:k + 1]
                cls = classes[t]
                if cls == "d":
                    junk = jdpool.tile([P, FEAT], F16, tag="jd")
                    nc.vector.scalar_tensor_tensor(
                        out=junk, in0=xt, scalar=1.0, in1=v_rep,
                        op0=MULT, op1=MULT, accum_out=acc)
                else:
                    prod = prpool.tile([P, FEAT], F16, tag="prod")
                    if cls == "g":
                        nc.gpsimd.tensor_tensor(out=prod, in0=xt, in1=v_gp,
                                                op=MULT)
                    else:
                        nc.vector.tensor_tensor(out=prod, in0=xt, in1=v_rep,
                                                op=MULT)
                    junk = japool.tile([P, FEAT], F16, tag="ja")
                    nc.scalar.activation(out=junk, in_=prod, func=COPY,
                                         accum_out=acc)
                if split and k % (TPB // 4) == TPB // 4 - 1 and k < TPB - 1:
                    tail(slice(k + 1 - TPB // 4, k + 1))
            tail(slice(TPB - TPB // 4, TPB) if split else slice(0, TPB))
            return xts, econ

        group_state = {}

        def emit_pool(bu, xts, econ):
            """PE pooling + normalize + transposes for a scored batch."""
            g, bi = divmod(bu, BPG)
            if bi == 0:
                pt_new = ps_pt.tile([P, FCH, P], F16, tag="pt", name="pt")
                group_state[g] = pt_new
            pt_ps = group_state[g]
            ebb = eb_all[:, bu, :]
            bp = ps_bp.tile([GPB, FEAT], F32, tag="bp")
            den = ps_den.tile([GPB, 1], F32, tag="den")
            for k in range(TPB):
                lhs = ebb[:, EBW * k:EBW * k + GPB]
                nc.tensor.matmul(bp, lhsT=lhs, rhs=xts[k],
                                 start=(k == 0), stop=(k == TPB - 1))
            nc.tensor.matmul(den, lhsT=econ, rhs=ones_col,
                             start=True, stop=True)
            denr = drpool.tile([GPB, 1], F32, tag="denr")
            nc.vector.reciprocal(denr, den)
            # normalize during the PSUM->SBUF copy
            pooled = plpool.tile([GPB, FEAT], F16, tag="pooled")
            nc.scalar.activation(out=pooled, in_=bp, func=COPY, scale=denr)
            # transposed pooled chunks collect in one accumulation group
            for c in range(FCH):
                nc.tensor.matmul(
                    pt_ps[:, c, bi * GPB:(bi + 1) * GPB],
                    lhsT=pooled[:, c * P:(c + 1) * P],
                    rhs=ident_sb, is_transpose=True,
                    start=(bi == 0 and c == 0),
                    stop=(bi == BPG - 1 and c == FCH - 1))
            if bi == BPG - 1:
                pt_sb = ptsb.tile([P, FCH, P], F16, tag="pt_sb")
                nc.scalar.copy(out=pt_sb, in_=pt_ps)
                out_ps = ps_out.tile([P, FEAT], F32)
                for c in range(FCH):
                    nc.tensor.matmul(out_ps, lhsT=pt_sb[:, c, :],
                                     rhs=wt_sb[:, c, :],
                                     start=(c == 0), stop=(c == FCH - 1))
                out_sb = outp.tile([P, FEAT], F16, tag="out_sb")
                nc.scalar.copy(out=out_sb, in_=out_ps)
                # output rides the ACT DGE ring, not behind X loads
                nc.scalar.dma_start(out=out[g], in_=out_sb)

        # one-batch-deep software pipeline: batch bu's scores are emitted
        # before batch bu-1's PE work, so score ops never queue behind
        # PE-dependent copies on the shared engines.
        pending = None
        for bu in range(NB + 1):
            if bu < NB:
                scored = emit_scores(bu, split=(bu == NB - 1))
            if pending is not None:
                emit_pool(bu - 1, *pending)
            pending = scored if bu < NB else None
    nc.compile()
    return nc


def _host_inputs(atomwise_output, W, att_weight):
    """Per-core input maps (host-side prep: fp16 casts + DMA-order packing)."""
    X = np.asarray(atomwise_output, dtype=np.float32)
    Wf = np.asarray(W, dtype=np.float32)
    att = np.asarray(att_weight, dtype=np.float32)
    v = Wf.T @ att                                             # v = W.T @ att
    vrep = np.ascontiguousarray(
        np.broadcast_to(v.astype(np.float16), (P, FEAT)))
    # wt[p, c, fo] = W.T[128c+p, fo]
    wtp = np.ascontiguousarray(
        Wf.T.astype(np.float16).reshape(FCH, P, FEAT).transpose(1, 0, 2))
    mask4 = (np.arange(P)[:, None] // 32 == np.arange(GPT)[None, :]).astype(
        np.float16)
    ident = np.eye(GPB, dtype=np.float16)
    zeros = np.zeros((P, NB * EBCOLS), dtype=np.float16)
    Xh = X.astype(np.float16)
    in_maps = []
    for c in range(N_CORES):
        xc = Xh[c * NA_CORE:(c + 1) * NA_CORE]
        # [tile, partition, feat] -> [partition, tile, feat]
        xp = np.ascontiguousarray(
            xc.reshape(NT, P, FEAT).transpose(1, 0, 2)
        ).reshape(P, NT * FEAT)
        in_maps.append({"x": xp, "wt": wtp, "vrep": vrep, "mask4": mask4,
                       "ident": ident, "zeros": zeros})
    return in_maps


def _kernel_numpy_fallback(atomwise_output, n_atoms_i, W, att_weight):
    """Exact reference semantics in numpy (used only for non-uniform segments)."""
    X = np.asarray(atomwise_output, dtype=np.float32)
    n_at = np.asarray(n_atoms_i).astype(np.int64)
    W = np.asarray(W, dtype=np.float32)
    att = np.asarray(att_weight, dtype=np.float32)
    h = X @ W.T
    s = (att * h).sum(-1)
    s = np.where(s >= 0, s, NEG_SLOPE * s)
    seg = np.repeat(np.arange(len(n_at)), n_at)[:len(s)]
    ngr = len(n_at)
    smax = np.full(ngr, -np.inf, dtype=np.float32)
    np.maximum.at(smax, seg, s)
    e = np.exp(s - smax[seg])
    den = np.zeros(ngr, dtype=np.float32)
    np.add.at(den, seg, e)
    wgt = e / den[seg]
    outp = np.zeros((ngr, h.shape[1]), dtype=np.float32)
    np.add.at(outp, seg, wgt[:, None] * h)
    return outp


def _run_on_device(atomwise_output, W, att_weight):
    from concourse.bass_utils import run_bass_kernel_spmd

    if "nc" not in _CACHED:
        _CACHED["nc"] = _build_program()
    nc = _CACHED["nc"]
    in_maps = _host_inputs(atomwise_output, W, att_weight)
    res = run_bass_kernel_spmd(nc, in_maps, list(range(N_CORES)))
    return np.concatenate(
        [res.results[c]["out"].reshape(NG_CORE, FEAT).astype(np.float32)
         for c in range(N_CORES)], axis=0)


def _run_in_subprocess(atomwise_output, n_atoms_i, W, att_weight):
    """Last-resort retry in a fresh process: a transient
    NRT_EXEC_UNIT_UNRECOVERABLE wedges the current NRT client session, but a
    new process (fresh axon boot) recovers. Arrays go via a temp dir."""
    import os, subprocess, sys, tempfile
    kdir = os.path.dirname(os.path.abspath(__file__))
    with tempfile.TemporaryDirectory() as td:
        np.save(os.path.join(td, "x.npy"), np.asarray(atomwise_output))
        np.save(os.path.join(td, "n.npy"), np.asarray(n_atoms_i))
        np.save(os.path.join(td, "w.npy"), np.asarray(W))
        np.save(os.path.join(td, "a.npy"), np.asarray(att_weight))
        driver = (
            "import sys, os, numpy as np\n"
            f"sys.path.insert(0, {kdir!r})\n"
            "import kernel\n"
            f"td = {td!r}\n"
            "out = kernel.kernel(np.load(td+'/x.npy'), np.load(td+'/n.npy'),\n"
            "                    np.load(td+'/w.npy'), np.load(td+'/a.npy'))\n"
            "np.save(td+'/out.npy', out)\n"
        )
        env = dict(os.environ, KERNEL_NO_SUBPROC="1")
        subprocess.run([sys.executable, "-c", driver], env=env, check=True,
                       timeout=1800)
        return np.load(os.path.join(td, "out.npy"))


def kernel(atomwise_output, n_atoms_i, W, att_weight):
    import os
    n_at = np.asarray(n_atoms_i)
    uniform = (
        atomwise_output.shape == (N_ATOMS, FEAT)
        and n_at.shape == (N_GRAPHS,)
        and np.all(n_at == N_ATOMS // N_GRAPHS)
    )
    if not uniform:
        return _kernel_numpy_fallback(atomwise_output, n_atoms_i, W, att_weight)

    try:
        out = _run_on_device(atomwise_output, W, att_weight)
    except Exception:
        try:
            out = _run_on_device(atomwise_output, W, att_weight)
        except Exception:
            if os.environ.get("KERNEL_NO_SUBPROC"):
                raise
            out = _run_in_subprocess(atomwise_output, n_atoms_i, W, att_weight)
    return out.astype(np.float32)
